# revision 16
# baseline (speedup 1.0000x reference)
"""Trainium2 Bass kernel for nn_CEM (concept embedding model).

Strategy (expert-parallel): shard the C=64 concept axis across 8 cores
(8 concepts/core). Each core reads x[:, :, c0:c0+8] (transposed on host to
[F, C_local, B] so DMA is contiguous), keeps all its per-concept weights
resident in SBUF, and computes everything in the "transposed" orientation
(feature/hidden dims on partitions, batch on the free axis). All matmuls run
as float32r (full fp32 storage, full-rate PE). The pos/neg blend
  final = neg + (pos - neg) * sigmoid(pred)
is computed with selector matmuls so no cross-partition element ops are
needed. Outputs are written in transposed per-core layouts and reassembled
on the host.
"""

import os

import numpy as np

B, F, C, H, E = 8192, 256, 64, 64, 16
MM_NAMES = {"xt", "wl1", "wl2", "wr1", "wdn", "wr2", "wr3",
            "sel1", "sel2", "finsel", "ones"}
NCORES = 8
CL = C // NCORES          # concepts per core
BC = 512                  # batch columns per chunk (= one fp32 PSUM bank)
NCHUNK = B // BC
NPAIR = CL // 2
NQUAD = CL // 4


# ---------------------------------------------------------------------------
# Host-side packing
# ---------------------------------------------------------------------------

def pack_core_inputs(core, x, pW1, pb1, pW2, pb2, pW3, pb3,
                     nW1, nb1, nW2, nb2, nW3, nb3,
                     rW1, rb1, rW2, rb2, rW3, rb3, xt_full=None):
    """Build the per-core input dict (all fp32 numpy arrays)."""
    f32 = np.float32
    c0 = core * CL
    sl = slice(c0, c0 + CL)

    if xt_full is None:
        xt_full = np.ascontiguousarray(np.transpose(x, (1, 2, 0)))  # [F, C, B]
    xt = np.ascontiguousarray(xt_full[:, sl, :])                    # [F, CL, B]

    # L1 weights: lhsT [f, m] per (ftile, concept); m: 0-63 pos h, 64-127 neg h
    wl1 = np.zeros((128, 2, CL, 128), f32)
    for t in range(2):
        fs = slice(128 * t, 128 * (t + 1))
        wl1[:, t, :, 0:64] = np.transpose(pW1[sl, fs, :], (1, 0, 2))
        wl1[:, t, :, 64:128] = np.transpose(nW1[sl, fs, :], (1, 0, 2))
    bl1 = np.concatenate([pb1[sl].T, nb1[sl].T], axis=0).astype(f32)  # [128, CL]

    # L2: block-diag [h_in(pos|neg), c, m(pos|neg)]
    wl2 = np.zeros((128, CL, 128), f32)
    wl2[0:64, :, 0:64] = np.transpose(pW2[sl], (1, 0, 2))
    wl2[64:128, :, 64:128] = np.transpose(nW2[sl], (1, 0, 2))
    bl2 = np.concatenate([pb2[sl].T, nb2[sl].T], axis=0).astype(f32)

    # r1 folded through L3: rW1eff[c] = [pW3 @ rW1[:E]; nW3 @ rW1[E:]]
    # fp32r matmul dst must start at partition 0, so pack the pair into
    # zero-padded M=128 columns (even concept -> cols 0-63, odd -> 64-127)
    wr1 = np.zeros((128, CL, 128), f32)
    br1p = np.zeros((64, CL), f32)
    for j in range(CL):
        c = c0 + j
        lo = 0 if j % 2 == 0 else 64
        wr1[0:64, j, lo:lo + 64] = pW3[c] @ rW1[c][0:E, :]
        wr1[64:128, j, lo:lo + 64] = nW3[c] @ rW1[c][E:2 * E, :]
        br1p[:, j] = rb1[c] + pb3[c] @ rW1[c][0:E, :] + nb3[c] @ rW1[c][E:2 * E, :]
    # pair layout for the relu epilogue: rows 0-63 even concept, 64-127 odd
    br1 = np.zeros((128, NPAIR), f32)
    for pr in range(NPAIR):
        br1[0:64, pr] = br1p[:, 2 * pr]
        br1[64:128, pr] = br1p[:, 2 * pr + 1]

    # diff/neg head: cols 0-15 Wdiff (pos rows pW3, neg rows -nW3),
    #                cols 16-31 Wneg (neg rows nW3)
    wdn = np.zeros((128, CL, 128), f32)
    for cl_i in range(CL):
        lo = 32 * (cl_i % 4)
        wdn[0:64, cl_i, lo:lo + 16] = pW3[c0 + cl_i]
        wdn[64:128, cl_i, lo:lo + 16] = -nW3[c0 + cl_i]
        wdn[64:128, cl_i, lo + 16:lo + 32] = nW3[c0 + cl_i]
    # bias for psum_dn epilogue, per quad: rows 32j..32j+16 = pb3-nb3 of c=4q+j
    bdn = np.zeros((128, NQUAD), f32)
    for q in range(NQUAD):
        for j in range(4):
            c = c0 + 4 * q + j
            bdn[32 * j:32 * j + 16, q] = pb3[c] - nb3[c]

    # r2 pair block-diag
    wr2 = np.zeros((128, NPAIR, 128), f32)
    br2 = np.zeros((128, NPAIR), f32)
    for pr in range(NPAIR):
        wr2[0:64, pr, 0:64] = rW2[c0 + 2 * pr]
        wr2[64:128, pr, 64:128] = rW2[c0 + 2 * pr + 1]
        br2[0:64, pr] = rb2[c0 + 2 * pr]
        br2[64:128, pr] = rb2[c0 + 2 * pr + 1]

    # r3: per pair, M=4 one-hot cols (accumulated across the quad)
    wr3 = np.zeros((128, NPAIR, 32), f32)
    for pr in range(NPAIR):
        j_even = 2 * (pr % 2)
        wr3[0:64, pr, j_even] = rW3[c0 + 2 * pr][:, 0]
        wr3[64:128, pr, j_even + 1] = rW3[c0 + 2 * pr + 1][:, 0]
    br3 = np.zeros((4, NQUAD), f32)
    for q in range(NQUAD):
        for j in range(4):
            br3[j, q] = rb3[c0 + 4 * q + j, 0]

    # selectors (concept independent)
    sel1 = np.zeros((4, 128), f32)   # wv diff rows <- w_j
    sel2 = np.zeros((1, 128), f32)   # wv neg rows <- 1
    finsel = np.zeros((128, 2, 128), f32)
    for j in range(4):
        for e in range(16):
            sel1[j, 32 * j + e] = 1.0
            sel2[0, 32 * j + 16 + e] = 1.0
            for q in range(2):
                finsel[32 * j + e, q, 64 * q + 16 * j + e] = 1.0
                finsel[32 * j + 16 + e, q, 64 * q + 16 * j + e] = 1.0
    ones = np.ones((1, BC), f32)

    # final bias: row 16*cl + e = nb3[c0+cl, e]
    bfin = np.zeros((128, 1), f32)
    for cl_i in range(CL):
        bfin[16 * cl_i:16 * cl_i + 16, 0] = nb3[c0 + cl_i]

    return {
        "xt": xt, "wl1": wl1, "bl1": bl1, "wl2": wl2, "bl2": bl2,
        "wr1": wr1, "br1": br1, "wdn": wdn, "bdn": bdn,
        "wr2": wr2, "br2": br2, "wr3": wr3, "br3": br3,
        "sel1": sel1, "sel2": sel2, "finsel": finsel, "ones": ones,
        "bfin": bfin,
    }


def core_forward_numpy(inp):
    """Numpy golden model mirroring the device dataflow op-for-op."""
    f32 = np.float32
    xt, wl1, bl1 = inp["xt"], inp["wl1"], inp["bl1"]
    outf = np.zeros((128, B), f32)
    outp = np.zeros((CL, B), f32)
    relu = lambda v: np.maximum(v, 0.0)

    for ch in range(NCHUNK):
        bs = slice(ch * BC, (ch + 1) * BC)
        x0 = xt[0:128, :, bs]     # [128, CL, BC]
        x1 = xt[128:256, :, bs]
        psfin = np.zeros((128, BC), f32)
        for q in range(NQUAD):
            psdn = np.zeros((128, BC), f32)
            pspred = np.zeros((32, BC), f32)
            rh2_pair = {}
            for j in range(4):
                cl_i = 4 * q + j
                ps = wl1[:, 0, cl_i, :].T @ x0[:, cl_i, :]
                ps = ps + wl1[:, 1, cl_i, :].T @ x1[:, cl_i, :]
                h1 = relu(ps + bl1[:, cl_i:cl_i + 1])
                h2 = relu(inp["wl2"][:, cl_i, :].T @ h1 + inp["bl2"][:, cl_i:cl_i + 1])
                pr = cl_i // 2
                half = slice(0, 64) if j % 2 == 0 else slice(64, 128)
                if j % 2 == 0:
                    rh2_pair["psr1"] = np.zeros((128, BC), f32)
                rh2_pair["psr1"] += inp["wr1"][:, cl_i, :].T @ h2
                psdn += inp["wdn"][:, cl_i, :].T @ h2
                if j % 2 == 1:
                    rh1 = relu(rh2_pair["psr1"] + inp["br1"][:, pr:pr + 1])
                    psr2 = inp["wr2"][:, pr, :].T @ rh1
                    rh2 = relu(psr2 + inp["br2"][:, pr:pr + 1])
                    pspred += inp["wr3"][:, pr, :].T @ rh2
            combq = psdn + inp["bdn"][:, q:q + 1]
            wsm = 1.0 / (1.0 + np.exp(-(pspred[0:4] + inp["br3"][:, q:q + 1])))
            predq = pspred[0:4] + inp["br3"][:, q:q + 1]
            outp[4 * q:4 * q + 4, bs] = predq
            pswv = inp["sel1"].T @ wsm + inp["sel2"].T @ inp["ones"]
            prodq = combq * pswv
            psfin += inp["finsel"][:, q, :].T @ prodq
        outf[:, bs] = psfin + inp["bfin"]
    return outf, outp


def gather_outputs(outf_list, outp_list):
    """Reassemble full outputs from per-core transposed results."""
    outf = np.stack(outf_list)                       # [8, 128, B]
    outf = outf.reshape(NCORES, CL, E, B)            # (core, cl, e, b)
    ff = np.ascontiguousarray(
        np.transpose(outf, (3, 2, 0, 1)).reshape(B, E * C)).astype(np.float32)
    outp = np.stack(outp_list)                       # [8, CL, B]
    pred = np.ascontiguousarray(
        np.transpose(outp, (2, 0, 1)).reshape(B, C)).astype(np.float32)
    return ff, pred


# ---------------------------------------------------------------------------
# Bass kernel
# ---------------------------------------------------------------------------

def build_bass():
    import sys
    if "/opt/trn_rl_repo" not in sys.path:
        sys.path.insert(0, "/opt/trn_rl_repo")
    import concourse.bass as bass
    import concourse.tile as tile
    from concourse import bacc, mybir
    from contextlib import ExitStack

    f32 = mybir.dt.float32
    f32r = mybir.dt.float32r
    AF = mybir.ActivationFunctionType
    ALU = mybir.AluOpType

    nc = bacc.Bacc("TRN2", target_bir_lowering=False, debug=False)

    dram = {}
    # matmul-feeding tensors are float32r (same 4-byte storage; PE runs the
    # reduced-precision full-rate path); biases and outputs stay float32
    specs = {
        "xt": (F, CL, B), "wl1": (128, 2, CL, 128), "bl1": (128, CL),
        "wl2": (128, CL, 128), "bl2": (128, CL),
        "wr1": (128, CL, 128), "br1": (128, NPAIR),
        "wdn": (128, CL, 128), "bdn": (128, NQUAD),
        "wr2": (128, NPAIR, 128), "br2": (128, NPAIR),
        "wr3": (128, NPAIR, 32), "br3": (4, NQUAD),
        "sel1": (4, 128), "sel2": (1, 128), "finsel": (128, 2, 128),
        "ones": (1, BC), "bfin": (128, 1),
    }
    mmdt = f32r if os.environ.get("CEM_DTYPE", "f32r") == "f32r" else mybir.dt.bfloat16
    dtypes = {name: (mmdt if name in MM_NAMES else f32) for name in specs}
    for name, shape in specs.items():
        dram[name] = nc.dram_tensor(name, list(shape), dtypes[name],
                                    kind="ExternalInput")
    OUTF = nc.dram_tensor("outf", [128, B], f32, kind="ExternalOutput")
    OUTP = nc.dram_tensor("outp", [CL, B], f32, kind="ExternalOutput")

    def r(ap):
        return ap

    ablate = int(os.environ.get("CEM_ABLATE", "5"))
    with tile.TileContext(nc) as tc, ExitStack() as ctx:
        const = ctx.enter_context(tc.tile_pool(name="const", bufs=1))
        sb = {}
        for name, shape in specs.items():
            if name == "xt":
                continue
            t = const.tile(list(shape), dtypes[name], tag=name)
            nc.sync.dma_start(t[:], dram[name][:])
            sb[name] = t

        xpool = ctx.enter_context(tc.tile_pool(name="x", bufs=2))
        hpool = ctx.enter_context(tc.tile_pool(name="h", bufs=2))
        rpool = ctx.enter_context(tc.tile_pool(name="r", bufs=2))
        qpool = ctx.enter_context(tc.tile_pool(name="q", bufs=2))
        fpool = ctx.enter_context(tc.tile_pool(name="f", bufs=2))
        ppool = ctx.enter_context(tc.tile_pool(name="p", bufs=2))

        psA = ctx.enter_context(tc.tile_pool(name="psA", bufs=3, space="PSUM"))
        psR1 = ctx.enter_context(tc.tile_pool(name="psR1", bufs=1, space="PSUM"))
        psD = ctx.enter_context(tc.tile_pool(name="psD", bufs=1, space="PSUM"))
        psP = ctx.enter_context(tc.tile_pool(name="psP", bufs=1, space="PSUM"))
        psF = ctx.enter_context(tc.tile_pool(name="psF", bufs=1, space="PSUM"))

        for ch in range(NCHUNK):
            bs = bass.ds(ch * BC, BC)
            x0 = xpool.tile([128, CL, BC], mmdt, tag="x0")
            nc.sync.dma_start(x0[:], dram["xt"][0:128, :, bs])
            x1 = xpool.tile([128, CL, BC], mmdt, tag="x1")
            nc.sync.dma_start(x1[:], dram["xt"][128:256, :, bs])

            psfin = psF.tile([128, BC], f32, tag="fin")
            for q in range(NQUAD):
                psdn = psD.tile([128, BC], f32, tag="dn")
                pspred = psP.tile([32, BC], f32, tag="pred")
                psr1 = None
                for j in range(4):
                    cl_i = 4 * q + j
                    pr = cl_i // 2
                    psh1 = psA.tile([128, BC], f32, tag="ps")
                    nc.tensor.matmul(psh1[:], lhsT=r(sb["wl1"][:, 0, cl_i, :]),
                                     rhs=r(x0[:, cl_i, :]), start=True, stop=False)
                    nc.tensor.matmul(psh1[:], lhsT=r(sb["wl1"][:, 1, cl_i, :]),
                                     rhs=r(x1[:, cl_i, :]), start=False, stop=True)
                    h1 = hpool.tile([128, BC], mmdt, tag="h1")
                    nc.scalar.activation(h1[:], psh1[:], AF.Relu,
                                         bias=sb["bl1"][:, cl_i:cl_i + 1])
                    psh2 = psA.tile([128, BC], f32, tag="ps")
                    nc.tensor.matmul(psh2[:], lhsT=r(sb["wl2"][:, cl_i, :]),
                                     rhs=r(h1[:]), start=True, stop=True)
                    h2 = hpool.tile([128, BC], mmdt, tag="h2")
                    psh2_last = h2
                    nc.vector.tensor_scalar(h2[:], psh2[:],
                                            sb["bl2"][:, cl_i:cl_i + 1], 0.0,
                                            ALU.add, ALU.max)
                    if ablate < 3:
                        continue
                    if j % 2 == 0:
                        psr1 = psR1.tile([128, BC], f32, tag="r1")
                    nc.tensor.matmul(psr1[:], lhsT=r(sb["wr1"][:, cl_i, :]),
                                     rhs=r(h2[:]), start=(j % 2 == 0),
                                     stop=(j % 2 == 1))
                    nc.tensor.matmul(psdn[:], lhsT=r(sb["wdn"][:, cl_i, :]),
                                     rhs=r(h2[:]), start=(j == 0), stop=(j == 3))
                    if ablate < 4:
                        continue
                    if j % 2 == 1:
                        rh1 = rpool.tile([128, BC], mmdt, tag="rh1")
                        nc.scalar.activation(rh1[:], psr1[:], AF.Relu,
                                             bias=sb["br1"][:, pr:pr + 1])
                        psr2 = psA.tile([128, BC], f32, tag="ps")
                        nc.tensor.matmul(psr2[:], lhsT=r(sb["wr2"][:, pr, :]),
                                         rhs=r(rh1[:]), start=True, stop=True)
                        rh2 = rpool.tile([128, BC], mmdt, tag="rh2")
                        nc.vector.tensor_scalar(rh2[:], psr2[:],
                                                sb["br2"][:, pr:pr + 1], 0.0,
                                                ALU.add, ALU.max)
                        # single K=128 MM covers both concepts of the pair
                        # (f32r matmuls crash at runtime with non-zero base
                        # partition, so never slice the partition dim)
                        nc.tensor.matmul(pspred[:], lhsT=r(sb["wr3"][:, pr, :]),
                                         rhs=r(rh2[:]),
                                         start=(j == 1), stop=(j == 3))
                # quad tail
                if ablate < 3:
                    continue
                combq = qpool.tile([128, BC], f32, tag="combq")
                nc.scalar.activation(combq[:], psdn[:], AF.Identity,
                                     bias=sb["bdn"][:, q:q + 1])
                if ablate < 4 or os.environ.get("CEM_NOPRED") or os.environ.get("CEM_PREDONLY"):
                    continue
                wsm = qpool.tile([4, BC], mmdt, tag="wsm")
                sigf = AF.Relu if os.environ.get("CEM_NOSIG") else AF.Sigmoid
                nc.scalar.activation(wsm[:], pspred[0:4, :], sigf,
                                     bias=sb["br3"][:, q:q + 1])
                predq = ppool.tile([4, BC], f32, tag="predq")
                nc.vector.tensor_scalar_add(predq[:], pspred[0:4, :],
                                            sb["br3"][:, q:q + 1])
                nc.sync.dma_start(OUTP[4 * q:4 * q + 4, bs], predq[:])
                if ablate < 5:
                    continue
                pswv = psA.tile([128, BC], f32, tag="ps")
                nc.tensor.matmul(pswv[:], lhsT=r(sb["sel1"][:]), rhs=r(wsm[:]),
                                 start=True, stop=False)
                nc.tensor.matmul(pswv[:], lhsT=r(sb["sel2"][:]), rhs=r(sb["ones"][:]),
                                 start=False, stop=True)
                prodq = qpool.tile([128, BC], mmdt, tag="prodq")
                nc.vector.tensor_tensor(prodq[:], combq[:], pswv[:], ALU.mult)
                nc.tensor.matmul(psfin[:], lhsT=r(sb["finsel"][:, q, :]),
                                 rhs=r(prodq[:]),
                                 start=(q == 0), stop=(q == 1))
            finsb = fpool.tile([128, BC], f32, tag="finsb")
            if ablate >= 5:
                nc.scalar.activation(finsb[:], psfin[:], AF.Identity,
                                     bias=sb["bfin"][:, 0:1])
            else:
                nc.scalar.activation(finsb[:], psh2_last[:], AF.Identity, bias=0.0)
            nc.sync.dma_start(OUTF[:, bs], finsb[:])

    nc.compile()
    return nc


_NC_CACHE = {}


def kernel(**inputs):
    x = inputs["x"]
    args = [np.asarray(inputs[k], np.float32) for k in
            ["x", "pW1", "pb1", "pW2", "pb2", "pW3", "pb3",
             "nW1", "nb1", "nW2", "nb2", "nW3", "nb3",
             "rW1", "rb1", "rW2", "rb2", "rW3", "rb3"]]
    x = args[0]
    xt_full = np.ascontiguousarray(np.transpose(x, (1, 2, 0)))  # [F, C, B]
    in_maps = [pack_core_inputs(k, *args, xt_full=xt_full) for k in range(NCORES)]

    if os.environ.get("CEM_DTYPE", "f32r") == "bf16":
        import ml_dtypes
        for m in in_maps:
            for k in MM_NAMES:
                m[k] = m[k].astype(ml_dtypes.bfloat16)

    if os.environ.get("CEM_NUMPY", "0") == "1":
        results = [core_forward_numpy(m) for m in in_maps]
        return gather_outputs([r[0] for r in results], [r[1] for r in results])

    import sys
    if "/opt/trn_rl_repo" not in sys.path:
        sys.path.insert(0, "/opt/trn_rl_repo")
    from concourse.bass_utils import run_bass_kernel_spmd

    if "nc" not in _NC_CACHE:
        _NC_CACHE["nc"] = build_bass()
    nc = _NC_CACHE["nc"]

    trace = os.environ.get("CEM_TRACE", "0") == "1"
    res = run_bass_kernel_spmd(nc, in_maps, core_ids=list(range(NCORES)),
                               trace=trace)
    if trace:
        print("exec_time_ns:", res.exec_time_ns,
              "mean:", res.mean_exec_time_ns)
        _NC_CACHE["last_results"] = res
    outs = res.results
    return gather_outputs([o["outf"] for o in outs], [o["outp"] for o in outs])


# revision 19
# speedup vs baseline: 1.8694x; 1.8694x over previous
"""Trainium2 Bass kernel for nn_CEM (concept embedding model).

Strategy (expert-parallel): shard the C=64 concept axis across 8 cores
(8 concepts/core). Each core reads x[:, :, c0:c0+8] (transposed on host to
[F, C_local, B] so DMA is contiguous), keeps all its per-concept weights
resident in SBUF, and computes everything in the "transposed" orientation
(feature/hidden dims on partitions, batch on the free axis). All matmuls run
as float32r (full fp32 storage, full-rate PE). The pos/neg blend
  final = neg + (pos - neg) * sigmoid(pred)
is computed with selector matmuls so no cross-partition element ops are
needed. Outputs are written in transposed per-core layouts and reassembled
on the host.
"""

import os

import numpy as np

B, F, C, H, E = 8192, 256, 64, 64, 16
MM_NAMES = {"xt", "wl1", "wl2", "wr1", "wdn", "wr2", "wr3",
            "sel1", "sel2", "finsel", "ones"}
NCORES = 8
CL = C // NCORES          # concepts per core
BC = 512                  # batch columns per chunk (= one fp32 PSUM bank)
NCHUNK = B // BC
NPAIR = CL // 2
NQUAD = CL // 4


# ---------------------------------------------------------------------------
# Host-side packing
# ---------------------------------------------------------------------------

def pack_core_inputs(core, x, pW1, pb1, pW2, pb2, pW3, pb3,
                     nW1, nb1, nW2, nb2, nW3, nb3,
                     rW1, rb1, rW2, rb2, rW3, rb3, xt_full=None):
    """Build the per-core input dict (all fp32 numpy arrays)."""
    f32 = np.float32
    c0 = core * CL
    sl = slice(c0, c0 + CL)

    if xt_full is None:
        xt_full = np.ascontiguousarray(np.transpose(x, (1, 2, 0)))  # [F, C, B]
    xt = np.ascontiguousarray(xt_full[:, sl, :])                    # [F, CL, B]

    # L1 weights: lhsT [f, m] per (ftile, concept); m: 0-63 pos h, 64-127 neg h
    wl1 = np.zeros((128, 2, CL, 128), f32)
    for t in range(2):
        fs = slice(128 * t, 128 * (t + 1))
        wl1[:, t, :, 0:64] = np.transpose(pW1[sl, fs, :], (1, 0, 2))
        wl1[:, t, :, 64:128] = np.transpose(nW1[sl, fs, :], (1, 0, 2))
    bl1 = np.concatenate([pb1[sl].T, nb1[sl].T], axis=0).astype(f32)  # [128, CL]

    # L2: block-diag [h_in(pos|neg), c, m(pos|neg)]
    wl2 = np.zeros((128, CL, 128), f32)
    wl2[0:64, :, 0:64] = np.transpose(pW2[sl], (1, 0, 2))
    wl2[64:128, :, 64:128] = np.transpose(nW2[sl], (1, 0, 2))
    bl2 = np.concatenate([pb2[sl].T, nb2[sl].T], axis=0).astype(f32)

    # r1 folded through L3: rW1eff[c] = [pW3 @ rW1[:E]; nW3 @ rW1[E:]]
    # fp32r matmul dst must start at partition 0, so pack the pair into
    # zero-padded M=128 columns (even concept -> cols 0-63, odd -> 64-127)
    wr1 = np.zeros((128, CL, 128), f32)
    br1p = np.zeros((64, CL), f32)
    for j in range(CL):
        c = c0 + j
        lo = 0 if j % 2 == 0 else 64
        wr1[0:64, j, lo:lo + 64] = pW3[c] @ rW1[c][0:E, :]
        wr1[64:128, j, lo:lo + 64] = nW3[c] @ rW1[c][E:2 * E, :]
        br1p[:, j] = rb1[c] + pb3[c] @ rW1[c][0:E, :] + nb3[c] @ rW1[c][E:2 * E, :]
    # pair layout for the relu epilogue: rows 0-63 even concept, 64-127 odd
    br1 = np.zeros((128, NPAIR), f32)
    for pr in range(NPAIR):
        br1[0:64, pr] = br1p[:, 2 * pr]
        br1[64:128, pr] = br1p[:, 2 * pr + 1]

    # diff/neg head: cols 0-15 Wdiff (pos rows pW3, neg rows -nW3),
    #                cols 16-31 Wneg (neg rows nW3)
    wdn = np.zeros((128, CL, 128), f32)
    for cl_i in range(CL):
        lo = 32 * (cl_i % 4)
        wdn[0:64, cl_i, lo:lo + 16] = pW3[c0 + cl_i]
        wdn[64:128, cl_i, lo:lo + 16] = -nW3[c0 + cl_i]
        wdn[64:128, cl_i, lo + 16:lo + 32] = nW3[c0 + cl_i]
    # bias for psum_dn epilogue, per quad: rows 32j..32j+16 = pb3-nb3 of c=4q+j
    bdn = np.zeros((128, NQUAD), f32)
    for q in range(NQUAD):
        for j in range(4):
            c = c0 + 4 * q + j
            bdn[32 * j:32 * j + 16, q] = pb3[c] - nb3[c]

    # r2 pair block-diag
    wr2 = np.zeros((128, NPAIR, 128), f32)
    br2 = np.zeros((128, NPAIR), f32)
    for pr in range(NPAIR):
        wr2[0:64, pr, 0:64] = rW2[c0 + 2 * pr]
        wr2[64:128, pr, 64:128] = rW2[c0 + 2 * pr + 1]
        br2[0:64, pr] = rb2[c0 + 2 * pr]
        br2[64:128, pr] = rb2[c0 + 2 * pr + 1]

    # r3: per pair, M=4 one-hot cols (accumulated across the quad)
    wr3 = np.zeros((128, NPAIR, 32), f32)
    for pr in range(NPAIR):
        j_even = 2 * (pr % 2)
        wr3[0:64, pr, j_even] = rW3[c0 + 2 * pr][:, 0]
        wr3[64:128, pr, j_even + 1] = rW3[c0 + 2 * pr + 1][:, 0]
    br3 = np.zeros((4, NQUAD), f32)
    for q in range(NQUAD):
        for j in range(4):
            br3[j, q] = rb3[c0 + 4 * q + j, 0]

    # selectors (concept independent)
    sel1 = np.zeros((4, 128), f32)   # wv diff rows <- w_j
    sel2 = np.zeros((1, 128), f32)   # wv neg rows <- 1
    finsel = np.zeros((128, 64), f32)
    for j in range(4):
        for e in range(16):
            sel1[j, 32 * j + e] = 1.0
            sel2[0, 32 * j + 16 + e] = 1.0
            finsel[32 * j + e, 16 * j + e] = 1.0
            finsel[32 * j + 16 + e, 16 * j + e] = 1.0
    ones = np.ones((1, BC), f32)

    # final bias per quad: row 16*j + e = nb3[c0+4q+j, e]
    bfin = np.zeros((64, 2), f32)
    for q in range(2):
        for j in range(4):
            bfin[16 * j:16 * j + 16, q] = nb3[c0 + 4 * q + j]

    return {
        "xt": xt, "wl1": wl1, "bl1": bl1, "wl2": wl2, "bl2": bl2,
        "wr1": wr1, "br1": br1, "wdn": wdn, "bdn": bdn,
        "wr2": wr2, "br2": br2, "wr3": wr3, "br3": br3,
        "sel1": sel1, "sel2": sel2, "finsel": finsel, "ones": ones,
        "bfin": bfin,
    }


def core_forward_numpy(inp):
    """Numpy golden model mirroring the device dataflow op-for-op."""
    f32 = np.float32
    xt, wl1, bl1 = inp["xt"], inp["wl1"], inp["bl1"]
    outf = np.zeros((128, B), f32)
    outp = np.zeros((CL, B), f32)
    relu = lambda v: np.maximum(v, 0.0)

    for ch in range(NCHUNK):
        bs = slice(ch * BC, (ch + 1) * BC)
        x0 = xt[0:128, :, bs]     # [128, CL, BC]
        x1 = xt[128:256, :, bs]
        for q in range(NQUAD):
            psdn = np.zeros((128, BC), f32)
            pspred = np.zeros((32, BC), f32)
            rh2_pair = {}
            for j in range(4):
                cl_i = 4 * q + j
                ps = wl1[:, 0, cl_i, :].T @ x0[:, cl_i, :]
                ps = ps + wl1[:, 1, cl_i, :].T @ x1[:, cl_i, :]
                h1 = relu(ps + bl1[:, cl_i:cl_i + 1])
                h2 = relu(inp["wl2"][:, cl_i, :].T @ h1 + inp["bl2"][:, cl_i:cl_i + 1])
                pr = cl_i // 2
                half = slice(0, 64) if j % 2 == 0 else slice(64, 128)
                if j % 2 == 0:
                    rh2_pair["psr1"] = np.zeros((128, BC), f32)
                rh2_pair["psr1"] += inp["wr1"][:, cl_i, :].T @ h2
                psdn += inp["wdn"][:, cl_i, :].T @ h2
                if j % 2 == 1:
                    rh1 = relu(rh2_pair["psr1"] + inp["br1"][:, pr:pr + 1])
                    psr2 = inp["wr2"][:, pr, :].T @ rh1
                    rh2 = relu(psr2 + inp["br2"][:, pr:pr + 1])
                    pspred += inp["wr3"][:, pr, :].T @ rh2
            combq = psdn + inp["bdn"][:, q:q + 1]
            wsm = 1.0 / (1.0 + np.exp(-(pspred[0:4] + inp["br3"][:, q:q + 1])))
            predq = pspred[0:4] + inp["br3"][:, q:q + 1]
            outp[4 * q:4 * q + 4, bs] = predq
            pswv = inp["sel1"].T @ wsm + inp["sel2"].T @ inp["ones"]
            prodq = combq * pswv
            psfin_q = inp["finsel"].T @ prodq
            outf[64 * q:64 * (q + 1), bs] = psfin_q + inp["bfin"][:, q:q + 1]
    return outf, outp


def gather_outputs(outf_list, outp_list):
    """Reassemble full outputs from per-core transposed results."""
    outf = np.stack(outf_list)                       # [8, 128, B]
    outf = outf.reshape(NCORES, CL, E, B)            # (core, cl, e, b)
    ff = np.ascontiguousarray(
        np.transpose(outf, (3, 2, 0, 1)).reshape(B, E * C)).astype(np.float32)
    outp = np.stack(outp_list)                       # [8, CL, B]
    pred = np.ascontiguousarray(
        np.transpose(outp, (2, 0, 1)).reshape(B, C)).astype(np.float32)
    return ff, pred


# ---------------------------------------------------------------------------
# Bass kernel
# ---------------------------------------------------------------------------

def build_bass():
    import sys
    if "/opt/trn_rl_repo" not in sys.path:
        sys.path.insert(0, "/opt/trn_rl_repo")
    import concourse.bass as bass
    import concourse.tile as tile
    from concourse import bacc, mybir
    from contextlib import ExitStack

    f32 = mybir.dt.float32
    f32r = mybir.dt.float32r
    AF = mybir.ActivationFunctionType
    ALU = mybir.AluOpType

    nc = bacc.Bacc("TRN2", target_bir_lowering=False, debug=False)

    dram = {}
    # matmul-feeding tensors are float32r (same 4-byte storage; PE runs the
    # reduced-precision full-rate path); biases and outputs stay float32
    specs = {
        "xt": (F, CL, B), "wl1": (128, 2, CL, 128), "bl1": (128, CL),
        "wl2": (128, CL, 128), "bl2": (128, CL),
        "wr1": (128, CL, 128), "br1": (128, NPAIR),
        "wdn": (128, CL, 128), "bdn": (128, NQUAD),
        "wr2": (128, NPAIR, 128), "br2": (128, NPAIR),
        "wr3": (128, NPAIR, 32), "br3": (4, NQUAD),
        "sel1": (4, 128), "sel2": (1, 128), "finsel": (128, 64),
        "ones": (1, BC), "bfin": (64, 2),
    }
    mmdt = f32r if os.environ.get("CEM_DTYPE", "f32r") == "f32r" else mybir.dt.bfloat16
    dtypes = {name: (mmdt if name in MM_NAMES else f32) for name in specs}
    for name, shape in specs.items():
        dram[name] = nc.dram_tensor(name, list(shape), dtypes[name],
                                    kind="ExternalInput")
    OUTF = nc.dram_tensor("outf", [128, B], f32, kind="ExternalOutput")
    OUTP = nc.dram_tensor("outp", [CL, B], f32, kind="ExternalOutput")

    def r(ap):
        return ap

    ablate = int(os.environ.get("CEM_ABLATE", "5"))
    with tile.TileContext(nc) as tc, ExitStack() as ctx:
        const = ctx.enter_context(tc.tile_pool(name="const", bufs=1))
        sb = {}
        for name, shape in specs.items():
            if name == "xt":
                continue
            t = const.tile(list(shape), dtypes[name], tag=name)
            nc.sync.dma_start(t[:], dram[name][:])
            sb[name] = t

        big = (mmdt != f32r)
        xpool = ctx.enter_context(tc.tile_pool(name="x", bufs=4 if big else 3))
        hpool = ctx.enter_context(tc.tile_pool(name="h", bufs=6 if big else 4))
        rpool = ctx.enter_context(tc.tile_pool(name="r", bufs=4 if big else 3))
        qpool = ctx.enter_context(tc.tile_pool(name="q", bufs=4 if big else 3))
        fpool = ctx.enter_context(tc.tile_pool(name="f", bufs=4))
        ppool = ctx.enter_context(tc.tile_pool(name="p", bufs=2))

        psH1 = ctx.enter_context(tc.tile_pool(name="psH1", bufs=2, space="PSUM"))
        psH2 = ctx.enter_context(tc.tile_pool(name="psH2", bufs=2, space="PSUM"))
        psM = ctx.enter_context(tc.tile_pool(name="psM", bufs=1, space="PSUM"))
        psR1 = ctx.enter_context(tc.tile_pool(name="psR1", bufs=1, space="PSUM"))
        psD = ctx.enter_context(tc.tile_pool(name="psD", bufs=1, space="PSUM"))
        psP = ctx.enter_context(tc.tile_pool(name="psP", bufs=1, space="PSUM"))

        for ch in range(NCHUNK):
            bs = bass.ds(ch * BC, BC)
            x0 = xpool.tile([128, CL, BC], mmdt, tag="x0")
            nc.sync.dma_start(x0[:], dram["xt"][0:128, :, bs])
            x1 = xpool.tile([128, CL, BC], mmdt, tag="x1")
            nc.sync.dma_start(x1[:], dram["xt"][128:256, :, bs])

            for q in range(NQUAD):
                psdn = psD.tile([128, BC], f32, tag="dn")
                pspred = psP.tile([32, BC], f32, tag="pred")
                psr1 = None
                for j in range(4):
                    cl_i = 4 * q + j
                    pr = cl_i // 2
                    psh1 = psH1.tile([128, BC], f32, tag="psh1")
                    nc.tensor.matmul(psh1[:], lhsT=r(sb["wl1"][:, 0, cl_i, :]),
                                     rhs=r(x0[:, cl_i, :]), start=True, stop=False)
                    nc.tensor.matmul(psh1[:], lhsT=r(sb["wl1"][:, 1, cl_i, :]),
                                     rhs=r(x1[:, cl_i, :]), start=False, stop=True)
                    h1 = hpool.tile([128, BC], mmdt, tag="h1")
                    nc.scalar.activation(h1[:], psh1[:], AF.Relu,
                                         bias=sb["bl1"][:, cl_i:cl_i + 1])
                    psh2 = psH2.tile([128, BC], f32, tag="psh2")
                    nc.tensor.matmul(psh2[:], lhsT=r(sb["wl2"][:, cl_i, :]),
                                     rhs=r(h1[:]), start=True, stop=True)
                    h2 = hpool.tile([128, BC], mmdt, tag="h2")
                    psh2_last = h2
                    nc.vector.tensor_scalar(h2[:], psh2[:],
                                            sb["bl2"][:, cl_i:cl_i + 1], 0.0,
                                            ALU.add, ALU.max)
                    if ablate < 3:
                        continue
                    if j % 2 == 0:
                        psr1 = psR1.tile([128, BC], f32, tag="r1")
                    nc.tensor.matmul(psr1[:], lhsT=r(sb["wr1"][:, cl_i, :]),
                                     rhs=r(h2[:]), start=(j % 2 == 0),
                                     stop=(j % 2 == 1))
                    nc.tensor.matmul(psdn[:], lhsT=r(sb["wdn"][:, cl_i, :]),
                                     rhs=r(h2[:]), start=(j == 0), stop=(j == 3))
                    if ablate < 4:
                        continue
                    if j % 2 == 1:
                        rh1 = rpool.tile([128, BC], mmdt, tag="rh1")
                        nc.scalar.activation(rh1[:], psr1[:], AF.Relu,
                                             bias=sb["br1"][:, pr:pr + 1])
                        psr2 = psM.tile([128, BC], f32, tag="psm")
                        nc.tensor.matmul(psr2[:], lhsT=r(sb["wr2"][:, pr, :]),
                                         rhs=r(rh1[:]), start=True, stop=True)
                        rh2 = rpool.tile([128, BC], mmdt, tag="rh2")
                        nc.vector.tensor_scalar(rh2[:], psr2[:],
                                                sb["br2"][:, pr:pr + 1], 0.0,
                                                ALU.add, ALU.max)
                        # single K=128 MM covers both concepts of the pair
                        # (f32r matmuls crash at runtime with non-zero base
                        # partition, so never slice the partition dim)
                        nc.tensor.matmul(pspred[:], lhsT=r(sb["wr3"][:, pr, :]),
                                         rhs=r(rh2[:]),
                                         start=(j == 1), stop=(j == 3))
                # quad tail
                if ablate < 3:
                    continue
                combq = qpool.tile([128, BC], f32, tag="combq")
                nc.scalar.activation(combq[:], psdn[:], AF.Identity,
                                     bias=sb["bdn"][:, q:q + 1])
                if ablate < 4 or os.environ.get("CEM_NOPRED") or os.environ.get("CEM_PREDONLY"):
                    continue
                wsm = qpool.tile([4, BC], mmdt, tag="wsm")
                sigf = AF.Relu if os.environ.get("CEM_NOSIG") else AF.Sigmoid
                nc.scalar.activation(wsm[:], pspred[0:4, :], sigf,
                                     bias=sb["br3"][:, q:q + 1])
                predq = ppool.tile([4, BC], f32, tag="predq")
                nc.vector.tensor_scalar_add(predq[:], pspred[0:4, :],
                                            sb["br3"][:, q:q + 1])
                nc.sync.dma_start(OUTP[4 * q:4 * q + 4, bs], predq[:])
                if ablate < 5:
                    continue
                pswv = psM.tile([128, BC], f32, tag="psm")
                nc.tensor.matmul(pswv[:], lhsT=r(sb["sel1"][:]), rhs=r(wsm[:]),
                                 start=True, stop=False)
                nc.tensor.matmul(pswv[:], lhsT=r(sb["sel2"][:]), rhs=r(sb["ones"][:]),
                                 start=False, stop=True)
                prodq = qpool.tile([128, BC], mmdt, tag="prodq")
                nc.vector.tensor_tensor(prodq[:], combq[:], pswv[:], ALU.mult)
                psfin = psM.tile([64, BC], f32, tag="psm")
                nc.tensor.matmul(psfin[:], lhsT=r(sb["finsel"][:]),
                                 rhs=r(prodq[:]), start=True, stop=True)
                finsb = fpool.tile([64, BC], f32, tag="finsb")
                nc.scalar.activation(finsb[:], psfin[:], AF.Identity,
                                     bias=sb["bfin"][:, q:q + 1])
                nc.sync.dma_start(OUTF[64 * q:64 * (q + 1), bs], finsb[:])
            if ablate < 5:
                finsb = fpool.tile([128, BC], f32, tag="finsb_ab")
                nc.scalar.activation(finsb[:], psh2_last[:], AF.Identity, bias=0.0)
                nc.sync.dma_start(OUTF[:, bs], finsb[:])

    nc.compile()
    return nc


_NC_CACHE = {}


def kernel(**inputs):
    x = inputs["x"]
    args = [np.asarray(inputs[k], np.float32) for k in
            ["x", "pW1", "pb1", "pW2", "pb2", "pW3", "pb3",
             "nW1", "nb1", "nW2", "nb2", "nW3", "nb3",
             "rW1", "rb1", "rW2", "rb2", "rW3", "rb3"]]
    x = args[0]
    xt_full = np.ascontiguousarray(np.transpose(x, (1, 2, 0)))  # [F, C, B]
    in_maps = [pack_core_inputs(k, *args, xt_full=xt_full) for k in range(NCORES)]

    if os.environ.get("CEM_DTYPE", "f32r") == "bf16":
        import ml_dtypes
        for m in in_maps:
            for k in MM_NAMES:
                m[k] = m[k].astype(ml_dtypes.bfloat16)

    if os.environ.get("CEM_NUMPY", "0") == "1":
        results = [core_forward_numpy(m) for m in in_maps]
        return gather_outputs([r[0] for r in results], [r[1] for r in results])

    import sys
    if "/opt/trn_rl_repo" not in sys.path:
        sys.path.insert(0, "/opt/trn_rl_repo")
    from concourse.bass_utils import run_bass_kernel_spmd

    if "nc" not in _NC_CACHE:
        _NC_CACHE["nc"] = build_bass()
    nc = _NC_CACHE["nc"]

    trace = os.environ.get("CEM_TRACE", "0") == "1"
    res = run_bass_kernel_spmd(nc, in_maps, core_ids=list(range(NCORES)),
                               trace=trace)
    if trace:
        print("exec_time_ns:", res.exec_time_ns,
              "mean:", res.mean_exec_time_ns)
        _NC_CACHE["last_results"] = res
    outs = res.results
    return gather_outputs([o["outf"] for o in outs], [o["outp"] for o in outs])
